# revision 45
# baseline (speedup 1.0000x reference)
"""GAT (3-layer, 4-head) message-passing kernel for 8 Trainium2 NeuronCores.

Strategy (graph/data parallel per the sharding hint):
  * Nodes are ranked by in-degree and dealt round-robin to the 8 cores, so
    every core owns ~E/8 edges and every 128-row block has near-uniform
    degree. Each core owns a contiguous slice of the permuted node table.
  * Per layer, each core computes its chunk of the fused node table
    T = x @ [W | W.as | W.ad]  (attention projections folded into W on the
    host), then an AllGather replicates the table to all cores.
  * Edges are stored as a padded CSR: dst-block of 128 nodes on partitions,
    incoming-edge slots on the free dim. One dma_gather per (block, window)
    fetches h|as rows of all sources. int16 gather indices only reach 32768
    rows, so two overlapping windows (base 0 and base NTOT-32768) cover the
    table; flexible middle sources balance the two windows' slot counts.
  * Pad slots point at a sentinel row with as = -1e9 -> exp underflows to 0.
  * Segment softmax runs per dst row along the free dim (exact max
    subtraction); leaky_relu(x) = 0.8*(0.25x + relu(x)) built from ACT ops.
  * Weighted aggregation = bf16 multiply + in-place halving-tree reduction
    over edge slots (DVE), normalization by 1/s at the dst level.
  * BatchNorm: per-core partial sums via ones-matmul + tiny AllReduce.
  * Global mean-pool: batch one-hot matmul per block; final [8,256] partial
    sums are combined on the host, which also applies the 256->64->4 MLP
    head (trivial FLOPs) and softmax/argmax.
"""
import numpy as np

import concourse.bass as bass
import concourse.mybir as mybir
from concourse import tile
from concourse.bacc import Bacc
from concourse.bass_utils import run_bass_kernel_spmd

F32 = mybir.dt.float32
BF16 = mybir.dt.bfloat16
I16 = mybir.dt.int16

NCORES = 8
H, C = 4, 64
HC = H * C          # 256
TWC = HC + 2 * H    # 264 useful table cols (h | as | ad)
TW = 384            # bf16 table row width (768B, multiple of 256B)
NEG = 0.2
EPS_BN = 1e-5


# ----------------------------------------------------------------- host plan
def _plan(edge_index, n_nodes, force_win=None):
    N = n_nodes
    nloc_real = N // NCORES
    nblk = (nloc_real + 127) // 128
    nloc = nblk * 128
    ntot = NCORES * nloc
    win = force_win if force_win else min(32768, ntot)
    hi_base = ntot - win
    sent_lo = nloc_real                      # core 0 spare row
    sent_hi = (NCORES - 1) * nloc + nloc_real

    src = np.concatenate([edge_index[0], np.arange(N, dtype=np.int64)])
    dst = np.concatenate([edge_index[1], np.arange(N, dtype=np.int64)])
    deg = np.bincount(dst, minlength=N)

    rank = np.argsort(-deg, kind="stable")
    perm = np.empty(N, np.int64)
    r = np.arange(N)
    perm[rank] = (r % NCORES) * nloc + (r // NCORES)

    psrc = perm[src]
    pdst = perm[dst]
    d_core = pdst // nloc
    d_loc = pdst % nloc

    order = np.lexsort((psrc, d_loc, d_core))
    s_core, s_loc, s_src = d_core[order], d_loc[order], psrc[order]

    row_key = s_core * nloc + s_loc
    deg_row = np.bincount(row_key, minlength=NCORES * nloc).reshape(NCORES, nloc)
    lo_ok = s_src < win                       # fits window [0, win)
    hi_ok = s_src >= hi_base                  # fits window [hi_base, ntot)
    mlo_row = np.bincount(row_key[~hi_ok], minlength=NCORES * nloc).reshape(NCORES, nloc)
    mhi_row = np.bincount(row_key[~lo_ok], minlength=NCORES * nloc).reshape(NCORES, nloc)

    Ka = np.zeros(nblk, np.int64)
    Kh = np.zeros(nblk, np.int64)
    for b in range(nblk):
        sl = slice(b * 128, (b + 1) * 128)
        D = int(deg_row[:, sl].max())
        M1 = int(mlo_row[:, sl].max())
        M2 = int(mhi_row[:, sl].max())
        ka = max(M1, D - M2)
        Ka[b] = ka
        Kh[b] = max(M2, D - ka)

    idx_lo = [[np.full((128, int(Ka[b])), sent_lo, np.int64) for b in range(nblk)]
              for _ in range(NCORES)]
    idx_hi = [[np.full((128, int(Kh[b])), sent_hi - hi_base, np.int64) for b in range(nblk)]
              for _ in range(NCORES)]

    bounds = np.searchsorted(row_key, np.arange(NCORES * nloc + 1))
    for core in range(NCORES):
        base = core * nloc
        for loc in range(nloc_real):
            k0, k1 = bounds[base + loc], bounds[base + loc + 1]
            if k0 == k1:
                continue
            ss = s_src[k0:k1]
            b, p = loc // 128, loc % 128
            lo_e = ss[ss < hi_base]           # must go lo
            hi_e = ss[ss >= win]              # must go hi
            fx = ss[(ss >= hi_base) & (ss < win)]
            d = len(ss)
            m, f = len(lo_e), len(fx)
            L = int(np.clip((d + 1) // 2, max(m, d - Kh[b]), min(m + f, Ka[b])))
            nf = L - m
            la = np.concatenate([lo_e, fx[:nf]])
            ha = np.concatenate([fx[nf:], hi_e])
            idx_lo[core][b][p, :len(la)] = la
            idx_hi[core][b][p, :len(ha)] = ha - hi_base

    # pack: per block, order i = k*128 + p, wrap 16, replicate to 128 parts
    def pack(blocks):
        cols = []
        offs = []
        o = 0
        for a in blocks:        # a: [128, K]
            K = a.shape[1]
            offs.append(o)
            if K == 0:
                continue
            flat = a.T.reshape(-1)                    # i = k*128 + p
            buf = flat.reshape(-1, 16).T              # [16, 8K]
            cols.append(buf)
            o += buf.shape[1]
        if cols:
            packed = np.concatenate(cols, 1).astype(np.int16)
        else:
            packed = np.zeros((16, 0), np.int16)
        return np.tile(packed, (8, 1)), offs

    packed_lo, off_lo = [], None
    packed_hi, off_hi = [], None
    for core in range(NCORES):
        p_lo, off_lo = pack(idx_lo[core])
        p_hi, off_hi = pack(idx_hi[core])
        packed_lo.append(p_lo)
        packed_hi.append(p_hi)

    return dict(
        nloc_real=nloc_real, nblk=nblk, nloc=nloc, ntot=ntot, win=win,
        hi_base=hi_base, perm=perm, Ka=Ka, Kh=Kh, Kmax=int((Ka + Kh).max()),
        packed_lo=packed_lo, packed_hi=packed_hi, off_lo=off_lo, off_hi=off_hi,
        CL=packed_lo[0].shape[1], CH=packed_hi[0].shape[1],
    )


def _fold_wext(W, a_s, a_d):
    F = W.shape[0]
    Wr = W.reshape(F, H, C)
    We = np.zeros((F, TWC), np.float32)
    We[:, :HC] = W
    We[:, HC:HC + H] = np.einsum("fhc,hc->fh", Wr, a_s)
    We[:, HC + H:] = np.einsum("fhc,hc->fh", Wr, a_d)
    return We


# ------------------------------------------------------------- device kernel
def _build_nc(P, DIN):
    nblk, nloc, ntot = P["nblk"], P["nloc"], P["ntot"]
    nloc_real = P["nloc_real"]
    Ka, Kh, Kmax = P["Ka"], P["Kh"], P["Kmax"]
    CL, CH = P["CL"], P["CH"]
    hi_base = P["hi_base"]
    N = NCORES * nloc_real

    nc = Bacc()
    xct_p = nc.declare_dram_parameter("xct", [128, nloc], F32, isOutput=False)
    ilo_p = nc.declare_dram_parameter("idxlo", [128, max(CL, 1)], I16, isOutput=False)
    ihi_p = nc.declare_dram_parameter("idxhi", [128, max(CH, 1)], I16, isOutput=False)
    w0_p = nc.declare_dram_parameter("wext0", [DIN, TWC], F32, isOutput=False)
    w1_p = nc.declare_dram_parameter("wext1", [HC, TWC], F32, isOutput=False)
    w2_p = nc.declare_dram_parameter("wext2", [HC, TWC], F32, isOutput=False)
    gb_p = nc.declare_dram_parameter("gb", [1, 4 * HC], F32, isOutput=False)
    vm_p = nc.declare_dram_parameter("vmask", [128, nblk], F32, isOutput=False)
    bo_p = nc.declare_dram_parameter("bone", [128, nblk * 8], F32, isOutput=False)
    id_p = nc.declare_dram_parameter("ident", [128, 128], F32, isOutput=False)
    sm_p = nc.declare_dram_parameter("sentm", [128, 1], F32, isOutput=False)
    pool_p = nc.declare_dram_parameter("pooled", [8, HC], F32, isOutput=True)

    with tile.TileContext(nc) as tc:
        with (
            tc.tile_pool(name="dram", bufs=1, space="DRAM") as dram,
            tc.tile_pool(name="consts", bufs=1) as cp,
            tc.tile_pool(name="work", bufs=2) as wp,
            tc.tile_pool(name="gp", bufs=2) as gp,
            tc.tile_pool(name="whp", bufs=1) as whp,
            tc.tile_pool(name="psc", bufs=2, space="PSUM") as psc,
            tc.tile_pool(name="pst", bufs=2, space="PSUM") as pst,
            tc.tile_pool(name="psstat", bufs=1, space="PSUM") as psstat,
            tc.tile_pool(name="psbig", bufs=1, space="PSUM") as psbig,
        ):
            agin = dram.tile([nloc, TW], I16)
            tables = [dram.tile([ntot, TW], I16, addr_space="Shared",
                                name=f"table{i}") for i in range(3)]
            outdram = dram.tile([nloc, HC], F32)
            arin = [dram.tile([1, 2 * HC], F32, name=f"arin{i}") for i in range(2)]
            arout = [dram.tile([1, 2 * HC], F32, addr_space="Shared",
                               name=f"arout{i}") for i in range(2)]

            # ---- resident constants
            idx_lo_sb = cp.tile([128, max(CL, 1)], I16)
            idx_hi_sb = cp.tile([128, max(CH, 1)], I16)
            nc.sync.dma_start(idx_lo_sb[:], ilo_p[:])
            nc.sync.dma_start(idx_hi_sb[:], ihi_p[:])
            w_sb = []                                  # per layer: list of K-tiles
            for li, wp_ in enumerate((w0_p, w1_p, w2_p)):
                kt = []
                for kk in range(wp_.shape[0] // 128):
                    t = cp.tile([128, TWC], F32, name=f"w{li}_{kk}")
                    nc.sync.dma_start(t[:], wp_[kk * 128:(kk + 1) * 128, :])
                    kt.append(t)
                w_sb.append(kt)
            vm_sb = cp.tile([128, nblk], F32)
            nc.sync.dma_start(vm_sb[:], vm_p[:])
            bo_sb = cp.tile([128, nblk * 8], F32)
            nc.sync.dma_start(bo_sb[:], bo_p[:])
            id_sb = cp.tile([128, 128], F32)
            nc.sync.dma_start(id_sb[:], id_p[:])
            sm_sb = cp.tile([128, 1], F32)
            nc.sync.dma_start(sm_sb[:], sm_p[:])
            gb_sb = cp.tile([1, 4 * HC], F32)
            nc.sync.dma_start(gb_sb[:], gb_p[:])
            ones_sb = cp.tile([1, 128], F32)
            nc.vector.memset(ones_sb[:], 1.0)
            ad_all = cp.tile([128, nblk * H], F32)
            abc_sb = cp.tile([128, HC], F32, name="abc")   # BN scale bcast
            dbc_sb = cp.tile([128, HC], F32, name="dbc")   # BN shift bcast

            sent_p0 = nloc_real % 128                  # first spare partition

            # ---------------- chunk build for one layer
            def build_chunk(layer):
                for j in range(nblk):
                    pc = psc.tile([128, TWC], F32, name="pc", tag="pc")
                    if layer == 0:
                        xr_raw = wp.tile([128, 128], F32, name="xr_raw")
                        nc.sync.dma_start(xr_raw[:],
                                          xct_p[:, j * 128:(j + 1) * 128])
                        xr = wp.tile([128, 128], F32, name="xr")
                        nc.scalar.activation(xr[:], xr_raw[:],
                                             mybir.ActivationFunctionType.Relu)
                        nc.tensor.matmul(pc[:], xr[:], w_sb[0][0][:],
                                         start=True, stop=True)
                    else:
                        xin = wp.tile([128, HC], F32, name="xin")
                        nc.sync.dma_start(xin[:], outdram[j * 128:(j + 1) * 128, :])
                        xbn = wp.tile([128, HC], F32, name="xbn")
                        nc.vector.tensor_tensor(xbn[:], xin[:], abc_sb[:],
                                                mybir.AluOpType.mult)
                        nc.vector.tensor_tensor(xbn[:], xbn[:], dbc_sb[:],
                                                mybir.AluOpType.add)
                        xrl = wp.tile([128, HC], F32, name="xrl")
                        nc.scalar.activation(xrl[:], xbn[:],
                                             mybir.ActivationFunctionType.Relu)
                        for kk in range(2):
                            pt = pst.tile([128, 128], F32, name="pt", tag="pt")
                            nc.tensor.transpose(
                                pt[:], xrl[:, kk * 128:(kk + 1) * 128], id_sb[:])
                            xt = wp.tile([128, 128], F32, name="xt")
                            nc.vector.tensor_copy(xt[:], pt[:])
                            nc.tensor.matmul(pc[:], xt[:], w_sb[layer][kk][:],
                                             start=(kk == 0), stop=(kk == 1))
                    ck = wp.tile([128, TW], I16, name="ck")
                    ckf = ck.bitcast(F32)              # [128, 192]
                    nc.vector.memset(ck[:, HC + 2 * H * 2:TW], 0)
                    nc.vector.tensor_copy(ck.bitcast(BF16)[:, 0:HC], pc[:, 0:HC])
                    if j == nblk - 1:
                        nc.vector.tensor_scalar_add(ckf[:, 128:132],
                                                    pc[:, HC:HC + H],
                                                    sm_sb[:, 0:1])
                    else:
                        nc.vector.tensor_copy(ckf[:, 128:132], pc[:, HC:HC + H])
                    nc.vector.tensor_copy(ckf[:, 132:136], pc[:, HC + H:TWC])
                    nc.vector.tensor_copy(ad_all[:, j * H:(j + 1) * H],
                                          pc[:, HC + H:TWC])
                    nc.sync.dma_start(agin[j * 128:(j + 1) * 128, :], ck[:])

            # ---------------- edge phase for one layer
            import os as _os
            esub = int(_os.environ.get("GAT_EDGE", "99"))

            def edge_phase(layer, table, tail=True):
                if layer < 2:
                    ps_sum = psstat.tile([8, HC], F32, name="ps_sum", tag="stat_a")
                    ps_sum = ps_sum[0:1, :]
                    ps_sq = psstat.tile([1, HC], F32, name="ps_sq", tag="stat_b")
                else:
                    ps_pool = psstat.tile([8, HC], F32, name="ps_pool", tag="stat_a")
                for b in range(nblk):
                    ka, kh = int(Ka[b]), int(Kh[b])
                    K = ka + kh
                    g = gp.tile([128, Kmax, TW], I16, name="g")
                    if ka:
                        nc.gpsimd.dma_gather(
                            g[:, 0:ka, :], table[:],
                            idx_lo_sb[:, P["off_lo"][b]:P["off_lo"][b] + 8 * ka],
                            128 * ka, 128 * ka, TW, single_packet=False)
                    if kh:
                        nc.gpsimd.dma_gather(
                            g[:, ka:K, :], table[hi_base:, :],
                            idx_hi_sb[:, P["off_hi"][b]:P["off_hi"][b] + 8 * kh],
                            128 * kh, 128 * kh, TW, single_packet=False)
                    if esub < 2:
                        continue
                    gf = g.bitcast(F32)                 # [128, Kmax, 192]
                    gb16 = g.bitcast(BF16)              # [128, Kmax, 384]
                    pre = wp.tile([128, H, Kmax], F32, name="pre")
                    u = wp.tile([128, H, Kmax], F32, name="u")
                    mneg = wp.tile([128, H], F32, name="mneg")
                    m8 = wp.tile([128, H], F32, name="m8")
                    ex = wp.tile([128, H, Kmax], F32, name="ex")
                    exb = wp.tile([128, H, Kmax], BF16, name="exb")
                    s = wp.tile([128, H], F32, name="s")
                    rcp = wp.tile([128, H], F32, name="rcp")
                    AF = mybir.ActivationFunctionType
                    for h in range(H):
                        as_h = gf[:, 0:K, 128 + h].squeeze()       # [128, K]
                        nc.scalar.activation(pre[:, h, 0:K], as_h, AF.Identity,
                                             bias=ad_all[:, b * H + h:b * H + h + 1])
                        nc.scalar.activation(u[:, h, 0:K], pre[:, h, 0:K], AF.Relu)
                        nc.vector.scalar_tensor_tensor(
                            u[:, h, 0:K], pre[:, h, 0:K], 0.25, u[:, h, 0:K],
                            mybir.AluOpType.mult, mybir.AluOpType.add)
                        nc.vector.tensor_reduce(
                            mneg[:, h:h + 1], u[:, h, 0:K],
                            axis=mybir.AxisListType.X, op=mybir.AluOpType.max,
                            negate=True)
                    nc.vector.tensor_scalar_mul(m8[:], mneg[:], 0.8)
                    for h in range(H):
                        nc.scalar.activation(ex[:, h, 0:K], u[:, h, 0:K], AF.Exp,
                                             bias=m8[:, h:h + 1], scale=0.8,
                                             accum_out=s[:, h:h + 1])
                    nc.vector.reciprocal(rcp[:], s[:])
                    nc.vector.tensor_copy(exb[:, :, 0:K], ex[:, :, 0:K])
                    if esub < 3:
                        continue
                    wh = whp.tile([128, Kmax, HC], BF16, name="wh")
                    for h in range(H):
                        nc.vector.tensor_tensor(
                            wh[:, 0:K, h * C:(h + 1) * C],
                            gb16[:, 0:K, h * C:(h + 1) * C],
                            exb[:, h:h + 1, 0:K].rearrange("p o k -> p k o")
                               .broadcast_to([128, K, C]),
                            mybir.AluOpType.mult)
                    k = K
                    while k > 1:
                        h2 = (k + 1) // 2
                        nc.vector.tensor_tensor(
                            wh[:, 0:k - h2, :], wh[:, 0:k - h2, :],
                            wh[:, h2:k, :], mybir.AluOpType.add)
                        k = h2
                    if esub < 4:
                        continue
                    outb = wp.tile([128, HC], F32, name="outb")
                    for h in range(H):
                        nc.vector.tensor_scalar_mul(
                            outb[:, h * C:(h + 1) * C],
                            wh[:, 0, h * C:(h + 1) * C], rcp[:, h:h + 1])
                    if layer < 2:
                        nc.sync.dma_start(outdram[b * 128:(b + 1) * 128, :], outb[:])
                        sq = wp.tile([128, HC], F32, name="sq")
                        nc.scalar.activation(sq[:], outb[:], AF.Square)
                        nc.tensor.matmul(ps_sum[:], vm_sb[:, b:b + 1], outb[:],
                                         start=(b == 0), stop=(b == nblk - 1))
                        nc.tensor.matmul(ps_sq[:], vm_sb[:, b:b + 1], sq[:],
                                         start=(b == 0), stop=(b == nblk - 1))
                    else:
                        nc.tensor.matmul(ps_pool[:], bo_sb[:, b * 8:(b + 1) * 8],
                                         outb[:], start=(b == 0),
                                         stop=(b == nblk - 1))
                if not tail:
                    return
                if layer < 2:
                    st = wp.tile([1, 2 * HC], F32, name="st")
                    nc.vector.tensor_copy(st[:, 0:HC], ps_sum[:])
                    nc.vector.tensor_copy(st[:, HC:2 * HC], ps_sq[:])
                    nc.sync.dma_start(arin[layer][:], st[:])
                    nc.gpsimd.collective_compute(
                        "AllReduce", mybir.AluOpType.add,
                        ins=[arin[layer].opt()], outs=[arout[layer].opt()],
                        replica_groups=[list(range(NCORES))])
                    ar = wp.tile([1, 2 * HC], F32, name="ar")
                    nc.sync.dma_start(ar[:], arout[layer][:])
                    # A = gamma*rsqrt(var+eps); D = beta - mu*A  (rows [1,256])
                    mu = wp.tile([1, HC], F32, name="mu")
                    nc.vector.tensor_scalar_mul(mu[:], ar[:, 0:HC], 1.0 / N)
                    va = wp.tile([1, HC], F32, name="va")
                    nc.vector.tensor_scalar_mul(va[:], ar[:, HC:2 * HC], 1.0 / N)
                    mu2 = wp.tile([1, HC], F32, name="mu2")
                    nc.vector.tensor_tensor(mu2[:], mu[:], mu[:],
                                            mybir.AluOpType.mult)
                    nc.vector.tensor_tensor(va[:], va[:], mu2[:],
                                            mybir.AluOpType.subtract)
                    nc.vector.tensor_scalar_add(va[:], va[:], EPS_BN)
                    sd = wp.tile([1, HC], F32, name="sd")
                    nc.scalar.activation(sd[:], va[:],
                                         mybir.ActivationFunctionType.Sqrt)
                    rs = wp.tile([1, HC], F32, name="rs")
                    nc.vector.reciprocal(rs[:], sd[:])
                    arow = wp.tile([1, HC], F32, name="arow")
                    nc.vector.tensor_tensor(
                        arow[:], rs[:],
                        gb_sb[:, 2 * layer * HC:(2 * layer + 1) * HC],
                        mybir.AluOpType.mult)
                    drow = wp.tile([1, HC], F32, name="drow")
                    nc.vector.tensor_tensor(drow[:], mu[:], arow[:],
                                            mybir.AluOpType.mult)
                    nc.vector.tensor_tensor(
                        drow[:],
                        gb_sb[:, (2 * layer + 1) * HC:(2 * layer + 2) * HC],
                        drow[:], mybir.AluOpType.subtract)
                    pa = psbig.tile([128, HC], F32, name="pa", tag="pbc")
                    nc.tensor.matmul(pa[:], ones_sb[:], arow[:])
                    nc.vector.tensor_copy(abc_sb[:], pa[:])
                    pd = psbig.tile([128, HC], F32, name="pd", tag="pbc")
                    nc.tensor.matmul(pd[:], ones_sb[:], drow[:])
                    nc.vector.tensor_copy(dbc_sb[:], pd[:])
                else:
                    po = wp.tile([8, HC], F32, name="po")
                    nc.vector.tensor_copy(po[:], ps_pool[:])
                    nc.sync.dma_start(pool_p[:], po[:])

            def allgather(table):
                nc.gpsimd.collective_compute(
                    "AllGather", mybir.AluOpType.bypass,
                    ins=[agin.opt()], outs=[table.opt()],
                    replica_groups=[list(range(NCORES))])

            import os
            stage = int(os.environ.get("GAT_STAGE", "99"))
            # stage: 1=build0, 2=+ag0, 3=+edge0 blocks, 4=+bn0 tail,
            #        5=+build1, 6=+ag1, 7=+edge1 blocks, 8=+bn1,
            #        9=+build2, 10=+ag2, 11=+edge2 blocks+pool, 99=full
            step = 0
            done_pool = False
            for layer in range(3):
                step += 1
                if step > stage:
                    break
                build_chunk(layer)
                step += 1
                if step > stage:
                    break
                allgather(tables[layer])
                step += 1
                if step > stage:
                    break
                tail = (step + 1) <= stage
                edge_phase(layer, tables[layer], tail=tail)
                if layer == 2 and tail:
                    done_pool = True
                step += 1
                if step > stage:
                    break
            if not done_pool:
                po0 = wp.tile([8, HC], F32, name="po0")
                nc.vector.memset(po0[:], 0.0)
                nc.gpsimd.dma_start(pool_p[:], po0[:])

    nc.compile()
    return nc


# ------------------------------------------------------------------ host run
def _prepare(x, edge_index, batch, weights, DIN, P):
    nloc, nblk, ntot = P["nloc"], P["nblk"], P["ntot"]
    nloc_real = P["nloc_real"]
    perm = P["perm"]

    wext = [_fold_wext(weights["W0"], weights["as0"], weights["ad0"]),
            _fold_wext(weights["W1"], weights["as1"], weights["ad1"]),
            _fold_wext(weights["W2"], weights["as2"], weights["ad2"])]
    gb = np.concatenate([weights["g0"], weights["be0"], weights["g1"],
                         weights["be1"]]).astype(np.float32)[None, :]

    xp = np.zeros((ntot, DIN), np.float32)
    xp[perm] = x
    batch_p = np.full(ntot, -1, np.int64)
    batch_p[perm] = batch

    sentm = np.zeros((128, 1), np.float32)
    sentm[nloc_real % 128:] = -1.0e9
    in_maps = []
    for core in range(NCORES):
        sl = slice(core * nloc, (core + 1) * nloc)
        xct = np.ascontiguousarray(xp[sl].T)                       # [128, nloc]
        bp = batch_p[sl]
        bone = np.zeros((128, nblk * 8), np.float32)
        vmask = np.zeros((128, nblk), np.float32)
        for b in range(nblk):
            ids = bp[b * 128:(b + 1) * 128]
            for p in range(128):
                if ids[p] >= 0:
                    bone[p, b * 8 + ids[p]] = 1.0
                    vmask[p, b] = 1.0
        in_maps.append({
            "xct": xct,
            "idxlo": P["packed_lo"][core] if P["CL"] else np.zeros((128, 1), np.int16),
            "idxhi": P["packed_hi"][core] if P["CH"] else np.zeros((128, 1), np.int16),
            "wext0": wext[0], "wext1": wext[1], "wext2": wext[2],
            "gb": gb, "vmask": vmask, "bone": bone,
            "ident": np.eye(128, dtype=np.float32),
            "sentm": sentm,
        })

    nc = _build_nc(P, DIN)
    return nc, in_maps


def _postprocess(results, batch, weights):
    pooled = np.zeros((8, HC), np.float32)
    for core in range(NCORES):
        pooled += results[core]["pooled"]
    cnts = np.maximum(np.bincount(batch, minlength=8), 1).astype(np.float32)
    pooled = pooled / cnts[:, None] + weights["b2"]
    hid = pooled @ weights["fc1_w"] + weights["fc1_b"]
    logits = (hid @ weights["fc2_w"] + weights["fc2_b"]).astype(np.float32)
    e = np.exp(logits - logits.max(1, keepdims=True))
    probs = e / e.sum(1, keepdims=True)
    yhat = logits.argmax(1)[:, None].astype(np.int32)
    return (logits, probs, yhat)


def _run(x, edge_index, batch, weights, DIN, trace=False, sim=False,
         force_win=None):
    P = _plan(edge_index.astype(np.int64), x.shape[0], force_win=force_win)
    nc, in_maps = _prepare(x, edge_index, batch, weights, DIN, P)
    if sim:
        from concourse import bass_interp
        ms = bass_interp.MultiCoreSim(nc, NCORES, num_workers=8)
        for c in range(NCORES):
            for k, v in in_maps[c].items():
                ms.cores[c].tensor(k)[:] = v
        ms.simulate()
        results = [{"pooled": ms.cores[c].tensor("pooled").copy()}
                   for c in range(NCORES)]

        class R:
            pass
        res = R()
        res.results = results
        res.exec_time_ns = None
    else:
        res = run_bass_kernel_spmd(nc, in_maps, list(range(NCORES)), trace=trace)
    return _postprocess(res.results, batch, weights), res


def kernel(**inputs):
    inp = {k: np.asarray(v) for k, v in inputs.items()}
    out, _ = _run(inp["x"], inp["edge_index"].astype(np.int64),
                  inp["batch"].astype(np.int64), inp, DIN=128)
    return out
